# revision 40
# baseline (speedup 1.0000x reference)
"""Trainium2 Bass kernel for ragged KeyQueryAttention pooling.

Math (per batch b):
    logits[t] = sum_l (x @ K)[t,l] * (x @ Q)[t,l],   t < len_b
    att = softmax(logits over valid t)
    out[b]    = sum_t att[t] * x[t, :] + bias        (sum att == 1)

Device strategy (8 NeuronCores, data-parallel over batch):
  - B=64 batches sorted by length (desc), grouped into 8 slots of 8;
    core i takes batch rank 8*j+i for slot j. One SPMD program whose
    per-slot chunk counts n_j = ceil(max_group_len/128) are compiled
    from the actual lengths (value-specialized, cached per n-tuple).
  - Host casts seq to fp16 and uploads TWO images per core: the
    natural layout X [128(t%128), chunk, d] and the pre-transposed
    XT [128(d), chunk, t%128], each [128, ntot*128] with large
    contiguous DMA lines. This removes all on-chip transposes and
    PSUM->SBUF staging copies. Host uploads kq = [K | Q] fp16.
  - Per 8-chunk group: 8 fp16 matmuls (lhsT = XT chunk straight from
    SBUF) -> g = [keys|queries] (PSUM fp32), one ScalarE copy of the
    queries half -> SBUF, one DVE multiply (keys from PSUM x queries
    from SBUF). Per 16-chunk pair: one DVE reduce -> logits columns.
    No GpSimd work on the per-chunk path.
  - Per slot: GpSimd +mask (-1e30), DVE row max,
    TensorE transpose + DVE reduce + broadcast matmul -> -max, ScalarE
    exp (bias=-max) -> p fp16 with fp32 row sums (zrow) as accum_out,
    then n accumulating matmuls (lhsT = x chunk, moving = p column)
    -> weighted-sum column [128, 1] in PSUM.
  - Output [128, 2*SLOTS] fp32: cols j = wsum_j, cols 8+j = zrow_j.
    Host: out[b] = wsum/sum(zrow) + bias, un-permute batches.
"""

import os
import numpy as np

import concourse.bass as bass
import concourse.bacc as bacc
import concourse.tile as tile
from concourse import mybir
from concourse.bass_utils import run_bass_kernel_spmd
from concourse.masks import make_identity

B, T, D, L = 64, 8192, 128, 64
NCORES = 8
SLOTS = B // NCORES
F32 = mybir.dt.float32
F16 = mybir.dt.float16
G = 8    # chunks per PSUM/Square group
PG = 16  # chunks per DMA group / logits pair

LAST_EXEC_NS = None  # filled when KQA_TRACE=1

_PROG_CACHE = {}


def _build_program(n_list):
    nc = bacc.Bacc()
    ntot = sum(n_list)
    offs = [sum(n_list[:j]) for j in range(SLOTS)]

    X = nc.declare_dram_parameter("X", [128, ntot * 128], F16, isOutput=False)
    XT = nc.declare_dram_parameter("XT", [128, ntot * 128], F16, isOutput=False)
    kq = nc.declare_dram_parameter("kq", [D, 2 * L], F16, isOutput=False)
    maskp = nc.declare_dram_parameter("mask", [128, ntot], F32, isOutput=False)
    outp = nc.declare_dram_parameter("out", [128, 2 * SLOTS], F32, isOutput=True)

    AF = mybir.ActivationFunctionType
    ALU = mybir.AluOpType
    AX = mybir.AxisListType

    with tile.TileContext(nc) as tc:
        with (
            tc.tile_pool(name="consts", bufs=1) as consts,
            tc.tile_pool(name="xgp", bufs=4) as xgp,
            tc.tile_pool(name="xtp", bufs=4) as xtp,
            tc.tile_pool(name="pairp", bufs=3) as pairp,
            tc.tile_pool(name="slotp", bufs=2) as slotp,
            tc.tile_pool(name="psG", bufs=3, space="PSUM") as psG,
            tc.tile_pool(name="psW", bufs=2, space="PSUM") as psW,
        ):
            identity16 = consts.tile([128, 128], F16)
            make_identity(nc, identity16)
            negrow = consts.tile([1, 128], F16)
            nc.vector.memset(negrow, -1.0)
            kq_sb = consts.tile([D, 2 * L], F16)
            mask_sb = consts.tile([128, ntot], F32)
            logits = consts.tile([128, ntot], F32)
            out_sb = consts.tile([128, 2 * SLOTS], F32)

            ngroups = [-(-n // PG) for n in n_list]  # logits pair groups
            # chunks per DMA tile: small tiles for slot 0 so compute starts
            # as soon as the first 256 KB lands, big tiles elsewhere
            dgs = [8 if j == 0 else 32 for j in range(SLOTS)]
            xg_tiles = [[] for _ in range(SLOTS)]
            xt_tiles = [[] for _ in range(SLOTS)]

            def emit_dma(j, kind):
                n, off = n_list[j], offs[j]
                dg = dgs[j]
                src, pool, tiles = (
                    (XT, xtp, xt_tiles) if kind == "tr" else (X, xgp, xg_tiles)
                )
                tag = ("xt" if kind == "tr" else "xg") + str(dg)
                for k in range(-(-n // dg)):
                    c0 = k * dg
                    w = min(dg, n - c0)
                    t_ = pool.tile(
                        [128, dg, 128], F16, tag=tag, bufs=(8 if dg == 8 else 4),
                        name=f"{tag}_{j}_{k}",
                    )
                    nc.sync.dma_start(
                        out=t_[:, 0:w, :],
                        in_=src[:, (off + c0) * 128 : (off + c0 + w) * 128],
                    )
                    tiles[j].append(t_)

            def emit_A_pair(j, pk):
                n, off = n_list[j], offs[j]
                dg = dgs[j]
                c0 = pk * PG
                wtot = min(PG, n - c0)
                # alternate which engine carries the keys*queries combine:
                # even pairs copy all of g as fp16 (ScalarE) enabling a 2x
                # DVE multiply; odd pairs copy only queries (fp32) and let
                # DVE read keys straight from PSUM at 1x.
                full = False
                prod = pairp.tile([128, PG, L], F16, tag="prod")
                if full:
                    gc = pairp.tile([128, PG, 2 * L], F16, tag="gc")
                else:
                    qc = pairp.tile([128, PG, L], F32, tag="qc")
                for half in range(2):
                    h0 = half * G
                    w = min(G, wtot - h0)
                    if w <= 0:
                        break
                    g_ps = psG.tile([128, G, 128], F32, tag="g")
                    for i in range(w):
                        c = c0 + h0 + i
                        nc.tensor.matmul(
                            g_ps[:, i, :],
                            xt_tiles[j][c // dg][:, c % dg, :],
                            kq_sb,
                            start=True,
                            stop=True,
                        )
                    if full:
                        nc.scalar.activation(
                            gc[:, h0 : h0 + w, :], g_ps[:, 0:w, :], AF.Copy
                        )
                        nc.vector.tensor_tensor(
                            prod[:, h0 : h0 + w, :],
                            gc[:, h0 : h0 + w, 0:L],
                            gc[:, h0 : h0 + w, L : 2 * L],
                            op=ALU.mult,
                        )
                    else:
                        nc.scalar.activation(
                            qc[:, h0 : h0 + w, :], g_ps[:, 0:w, L : 2 * L], AF.Copy
                        )
                        nc.vector.tensor_tensor(
                            prod[:, h0 : h0 + w, :],
                            g_ps[:, 0:w, 0:L],
                            qc[:, h0 : h0 + w, :],
                            op=ALU.mult,
                        )
                nc.vector.tensor_reduce(
                    logits[:, off + c0 : off + c0 + wtot],
                    prod[:, 0:wtot, :],
                    axis=AX.X,
                    op=ALU.add,
                )

            bstate = {}

            def emit_B_stage(j, s):
                """Stage s of slot j's softmax chain; deps resolved >=1 pair ago."""
                n, off = n_list[j], offs[j]
                st = bstate.setdefault(j, {})
                if s == 0:
                    st["lm2"] = slotp.tile([128, 64], F32, tag="lm2", name="lm2")
                    nc.gpsimd.tensor_tensor(
                        st["lm2"][:, 0:n],
                        logits[:, off : off + n],
                        mask_sb[:, off : off + n],
                        op=ALU.add,
                    )
                elif s == 1:
                    rmax = slotp.tile([128, 1], F32, tag="rmax", name="rmax")
                    nc.vector.tensor_reduce(
                        rmax, st["lm2"][:, 0:n], axis=AX.X, op=ALU.max
                    )
                    # clamp to keep the fp16 transpose free of +-inf
                    st["rmax16"] = slotp.tile([128, 1], F16, tag="rmax16", name="rx")
                    nc.vector.tensor_scalar_max(st["rmax16"], rmax, -60000.0)
                    st["misc"] = psW.tile([128, 256], F32, tag="misc", name="misc")
                    st["rmT"] = st["misc"][0:1, 128:192].bitcast(F16)
                    nc.tensor.transpose(st["rmT"], st["rmax16"], identity16)
                elif s == 2:
                    st["maxs"] = slotp.tile([1, 1], F16, tag="maxs", name="maxs")
                    nc.vector.tensor_reduce(
                        st["maxs"], st["rmT"], axis=AX.X, op=ALU.max
                    )
                    nc.tensor.matmul(
                        st["misc"][:, 1:2], negrow, st["maxs"], start=True, stop=True
                    )
                    st["negm"] = slotp.tile([128, 1], F32, tag="negm", name="negm")
                    nc.scalar.activation(st["negm"], st["misc"][:, 1:2], AF.Copy)
                elif s == 3:
                    st["p"] = slotp.tile([128, 64], F16, tag="p", name="p")
                    nc.scalar.activation(
                        st["p"][:, 0:n],
                        st["lm2"][:, 0:n],
                        AF.Exp,
                        bias=st["negm"],
                        scale=1.0,
                        accum_out=out_sb[:, SLOTS + j : SLOTS + j + 1],
                    )

            def emit_B_wacc_block(j, count):
                """Emit up to `count` weighted-sum matmuls for slot j; returns
                True when the slot is finished (outputs emitted)."""
                n = n_list[j]
                dg = dgs[j]
                st = bstate[j]
                c0 = st.setdefault("wc", 0)
                c1 = min(n, c0 + count)
                for c in range(c0, c1):
                    nc.tensor.matmul(
                        st["misc"][:, 0:1],
                        xg_tiles[j][c // dg][:, c % dg, :],
                        st["p"][:, c : c + 1],
                        start=(c == 0),
                        stop=(c == n - 1),
                        skip_group_check=True,
                    )
                st["wc"] = c1
                if c1 < n:
                    return False
                bstate.pop(j)
                nc.scalar.activation(out_sb[:, j : j + 1], st["misc"][:, 0:1], AF.Copy)
                # stream this slot's two output columns out now
                nc.sync.dma_start(
                    out=outp[:, j :: SLOTS], in_=out_sb[:, j :: SLOTS]
                )
                return True

            # startup: weights, then slot 0 (small tiles for a fast start),
            # then slot 1
            nc.sync.dma_start(out=kq_sb, in_=kq[:, :])
            emit_dma(0, "tr")
            emit_dma(0, "nat")
            nc.sync.dma_start(out=mask_sb, in_=maskp[:, :])
            if SLOTS > 1:
                emit_dma(1, "tr")
                emit_dma(1, "nat")

            stage_pair = {0: 0, 1: 1, 2: 2, 3: 2}
            for j in range(SLOTS):
                nb = ngroups[j]
                for pk in range(nb):
                    emit_A_pair(j, pk)
                    if j >= 1:
                        for s in range(4):
                            if min(stage_pair[s], nb - 1) == pk:
                                emit_B_stage(j - 1, s)
                        # spread the weighted-sum matmuls between pairs to
                        # keep the PE busy (HAM stays unthrottled)
                        if pk >= 3 and "p" in bstate.get(j - 1, {}):
                            emit_B_wacc_block(j - 1, 16)
                if j >= 1:
                    while (j - 1) in bstate:
                        emit_B_wacc_block(j - 1, 64)
                    if j + 1 < SLOTS:
                        emit_dma(j + 1, "tr")
                        emit_dma(j + 1, "nat")
            for s in range(4):
                emit_B_stage(SLOTS - 1, s)
            while (SLOTS - 1) in bstate:
                emit_B_wacc_block(SLOTS - 1, 64)
    nc.finalize()
    return nc


def kernel(seq, lengths, key_w, query_w, bias):
    global LAST_EXEC_NS
    seq = np.asarray(seq, dtype=np.float32)
    lengths_np = np.asarray(lengths).astype(np.int64)
    key_w = np.asarray(key_w, dtype=np.float32)
    query_w = np.asarray(query_w, dtype=np.float32)
    bias = np.asarray(bias, dtype=np.float32)

    order = np.argsort(-lengths_np, kind="stable")  # descending length
    n_list = []
    for j in range(SLOTS):
        grp = order[j * NCORES : (j + 1) * NCORES]
        n_list.append(max(1, int(-(-int(lengths_np[grp].max()) // 128))))
    key = tuple(n_list)
    if key not in _PROG_CACHE:
        _PROG_CACHE[key] = _build_program(n_list)
    nc = _PROG_CACHE[key]

    seq16 = seq.astype(np.float16)
    kqcat = np.concatenate([key_w, query_w], axis=1).astype(np.float16)

    in_maps = []
    for i in range(NCORES):
        xblocks = []
        xtblocks = []
        mblocks = []
        for j, n in enumerate(n_list):
            b = int(order[j * NCORES + i])
            blk = seq16[b, : n * 128, :].reshape(n, 128, 128)
            xblocks.append(blk.transpose(1, 0, 2).reshape(128, n * 128))
            xtblocks.append(blk.transpose(2, 0, 1).reshape(128, n * 128))
            lb = int(lengths_np[b])
            col = np.where(np.arange(n * 128) < lb, 0.0, -1e30).astype(np.float32)
            mblocks.append(col.reshape(n, 128).T)
        in_maps.append(
            {
                "X": np.ascontiguousarray(np.concatenate(xblocks, axis=1)),
                "XT": np.ascontiguousarray(np.concatenate(xtblocks, axis=1)),
                "kq": kqcat,
                "mask": np.ascontiguousarray(np.concatenate(mblocks, axis=1)),
            }
        )

    trace = os.environ.get("KQA_TRACE") == "1"
    res = run_bass_kernel_spmd(
        nc, in_maps, core_ids=list(range(NCORES)), trace=trace
    )
    LAST_EXEC_NS = res.exec_time_ns

    out = np.empty((B, D), dtype=np.float32)
    for i in range(NCORES):
        r = res.results[i]["out"]  # [128, 2*SLOTS]
        for j in range(SLOTS):
            b = int(order[j * NCORES + i])
            z = r[:, SLOTS + j].astype(np.float64).sum()
            out[b] = (r[:, j] / z).astype(np.float32) + bias
    return out
